# revision 9
# baseline (speedup 1.0000x reference)
"""Causal multi-head attention on 8 Trainium2 NeuronCores.

Problem: B=4, L=S=2048, D=1024, H=16 (E=64), fp32, causal mask.
Sharding: B x H tensor-parallel. Core k handles batch b=k//2 and heads
h in [(k%2)*8, (k%2)*8+8) -- a contiguous [2048, 512] column slice of
q/k/v. No cross-core communication.

v2 design notes (vs the fp32r baseline at ~230us):
  - QK^T in bf16: stationary gets FWL (2x faster LDWEIGHTS) and the two
    heads of a pair sit on partitions 0-63 / 64-127, so bass's implicit
    tile_position row-tiling runs both heads' matmuls concurrently.
    Host folds a 16/ln2 scale into q, so PSUM scores are s'' = 16*s/ln2.
  - exp is the real wall (ScalarE is the only table engine, 1 elem/lane/
    cycle).  Split it across two engines:
      * diagonal blocks (they alone feed the short causal rows, which
        have no averaging to hide error) run real exp on ACT:
        P = exp(s/8)/8 = exp(s''*ln2/128 - 3ln2), bf16 out.
      * ~9/16 of the off-diagonal blocks run on DVE as a Schraudolph
        bit-trick: bf16(bits) where bits = max(s'' + 15872, 0) written
        as uint16 -- one tensor_scalar per span, +-3% sawtooth error
        that averages out over long softmax rows (validated 3.4e-3
        graded err in numpy).
  - AV unchanged: P^T block stationary (bf16, FWL), rhs = [V|1] so the
    matmul also accumulates softmax row-sums in PSUM column 64.
  - Epilogue batched: one reciprocal [128,4] + one broadcast multiply
    [128,4,64] per (pair, quad, head); one contiguous [128,512] output
    DMA per (pair, quad); host reassembles the layout.
  - All DMAs are contiguous >=1KB-per-partition transfers.
"""

import os

os.environ.setdefault("MYCRO_LOCAL_CACHE", "1")

import numpy as np

import concourse.bass as bass
import concourse.mybir as mybir
import concourse.tile as tile
from concourse import bacc
from concourse.bass_utils import run_bass_kernel_spmd

F32 = mybir.dt.float32
BF16 = mybir.dt.bfloat16
U16 = mybir.dt.uint16

B, L, D, H = 4, 2048, 1024, 16
E = D // H               # 64
NCORES = 8
HLOC = H // 2            # 8 heads per core
DLOC = HLOC * E          # 512 local feature columns
NPAIR = HLOC // 2        # 4 head pairs
QUAD = 512               # q columns per PSUM tile (per head)
NQUAD = L // QUAD        # 4
BLK = 128
NBLK = L // BLK          # 16 key blocks
NEG = -3.0e7             # additive mask in s'' units; exp -> 0
LN2 = float(np.log(2.0))
QSCALE = 16.0 / LN2      # host-side q scale: scores become s'' = 16 s/ln2
ACT_SCALE = LN2 / 128.0  # exp(ACT_SCALE*s'' - 3ln2) = exp(s/8)/8
ACT_BIAS = -3.0 * LN2
DVE_B = 15872.0          # bf16 bit-space bias: 128*(127-3)=15872
SKEW = 5                 # AV deferral (items) to hide exp latency
GRP = 2                  # items per PE batch: halves QK<->AV array switches
DVE16 = int(os.environ.get("KERNEL_DVE16", "9"))  # of 16 non-diag exps on DVE

last_exec_time_ns = None
last_results = None


def _build(mode: str) -> bass.Bass:
    """mode: 'causal' | 'none' | 'mask'."""
    nc = bacc.Bacc()

    qTd = nc.declare_dram_parameter("qT", [NPAIR, BLK, L], BF16, isOutput=False)
    kTd = nc.declare_dram_parameter("kT", [NPAIR, BLK, L], BF16, isOutput=False)
    v2d = nc.declare_dram_parameter("v2", [NPAIR, BLK, NBLK, 130], BF16, isOutput=False)
    ltd = nc.declare_dram_parameter("ltm", [BLK, BLK], BF16, isOutput=False)
    if mode == "mask":
        maskd = nc.declare_dram_parameter("mask", [L, L], F32, isOutput=False)
    outd = nc.declare_dram_parameter(
        "out", [NPAIR, NQUAD, BLK, 2, 4, 65], F32, isOutput=True
    )

    with tile.TileContext(nc) as tc:
        with (
            tc.tile_pool(name="singles", bufs=1) as singles,
            tc.tile_pool(name="stage", bufs=2) as stage,   # mask staging
            tc.tile_pool(name="tbig", bufs=2) as tbig,     # qT/kT/v2
            tc.tile_pool(name="ptp", bufs=SKEW + 4) as ptp,
            tc.tile_pool(name="epi", bufs=3) as epi,
            tc.tile_pool(name="psS", bufs=2, space="PSUM") as psS,
            tc.tile_pool(name="psO", bufs=2, space="PSUM") as psO,
        ):
            ltm = singles.tile([BLK, BLK], BF16)
            bias_t = singles.tile([BLK, 1], F32)
            nc.gpsimd.memset(bias_t[:, :], ACT_BIAS)
            warm = singles.tile([BLK, 1], F32)
            nc.gpsimd.memset(warm[:, :], 0.0)
            # fires the one-time ACT exp-table load while inputs stream in
            nc.scalar.activation(
                warm[:, :], warm[:, :], mybir.ActivationFunctionType.Exp,
                bias=0.0, scale=1.0,
            )

            def declare_inputs(p):
                # chunked loads; first k chunk is small so compute starts early
                kT = tbig.tile([BLK, L], BF16, tag="kT")
                qT = tbig.tile([BLK, L], BF16, tag="qT")
                v2 = tbig.tile([BLK, NBLK, 130], BF16, tag="v2")
                nc.scalar.dma_start(out=kT[:, 0:BLK], in_=kTd[p][:, 0:BLK])
                nc.sync.dma_start(out=qT[:, 0:256], in_=qTd[p][:, 0:256])
                nc.sync.dma_start(out=qT[:, 256:QUAD], in_=qTd[p][:, 256:QUAD])
                nc.sync.dma_start(out=kT[:, BLK:QUAD], in_=kTd[p][:, BLK:QUAD])
                nc.gpsimd.dma_start(out=v2[:, 0:4, :], in_=v2d[p][:, 0:4, :])
                for c in range(1, NQUAD):
                    sl = slice(c * QUAD, (c + 1) * QUAD)
                    nc.sync.dma_start(out=kT[:, sl], in_=kTd[p][:, sl])
                    nc.sync.dma_start(out=qT[:, sl], in_=qTd[p][:, sl])
                    jl = slice(c * 4, (c + 1) * 4)
                    nc.sync.dma_start(out=v2[:, jl, :], in_=v2d[p][:, jl, :])
                return (kT, qT, v2)

            # flat work list: (pair, quad, j); within a quad, spread the
            # cheap diagonal j's among the full-width ones so per-iteration
            # PE work stays level
            items = []
            for p in range(NPAIR):
                for Q in range(NQUAD):
                    jmax = 4 * (Q + 1) if mode == "causal" else NBLK
                    if mode == "causal":
                        nond = list(range(4 * Q))
                        order = []
                        for i in range(4):
                            order += nond[i * Q : (i + 1) * Q] + [4 * Q + i]
                    else:
                        order = list(range(jmax))
                    last_j = order[-1]
                    for j in order:
                        items.append((p, Q, j, last_j))

            av_queue = []   # deferred AV work
            epi_pend = []   # deferred epilogue steps
            quads = {}      # (p, Q) -> {"otn": [...], "onorm": tile, "rsb": [...]}

            def emit_epi(step):
                kind = step[0]
                if kind == "dma":
                    _, qd_, pp, QQ, h = step
                    nc.sync.dma_start(
                        out=outd[pp][QQ][:, h], in_=qd_["onorm"][:, h]
                    )
                    return
                # "cp": move [128,4,65] (AV outputs + rowsums) PSUM->SBUF on
                # whichever engine is less loaded; host does the divide.
                _, qd_, h = step
                fd = 4 * 65
                if load["dve"] + (fd + 120) / 0.96 < load["act"] + (fd + 313) / 1.2:
                    load["dve"] += (fd + 120) / 0.96
                    nc.vector.tensor_copy(qd_["onorm"][:, h], qd_["otn"][h])
                else:
                    load["act"] += (fd + 313) / 1.2
                    nc.scalar.activation(
                        qd_["onorm"][:, h],
                        qd_["otn"][h],
                        mybir.ActivationFunctionType.Copy,
                        bias=0.0,
                        scale=1.0,
                    )

            def emit_mask(it):
                # zero the invalid triangle of both heads after exp: bf16 0/1
                # multiply in SBUF (2x DVE mode, off the st WAR path).  Emitted
                # a couple of items before the AV that consumes pt so it never
                # head-of-line-blocks fresh exps in the DVE FIFO.
                if it[6] and not it[8]:
                    pt, t = it[4], it[7]
                    load["dve"] += (BLK * 2 / 2 + 58) / 0.96
                    pt3 = pt[:, :].rearrange("s (h w) -> s h w", h=2)
                    nc.vector.tensor_tensor(
                        out=pt3[:, :, t : t + BLK],
                        in0=pt3[:, :, t : t + BLK],
                        in1=ltm[:, :].unsqueeze(1).broadcast_to([BLK, 2, BLK]),
                        op=mybir.AluOpType.mult,
                    )
                    it[8] = True

            def emit_av(it):
                emit_mask(it)
                p, Q, j, last_j, pt, v2 = it[:6]
                qd_ = quads[(p, Q)]
                qb0 = max(0, j - 4 * Q) if mode == "causal" else 0
                for h in range(2):
                    for qb in range(qb0, 4):
                        c = h * QUAD + qb * BLK
                        nc.tensor.matmul(
                            qd_["otn"][h][:, qb, :],
                            lhsT=pt[:, c : c + BLK],
                            rhs=v2[:, j, h * 65 : (h + 1) * 65],
                            start=not qd_["started"][h],
                            stop=(j == last_j and qb == 3),
                        )
                        qd_["started"][h] = True
                if j == last_j:
                    for h in range(2):
                        epi_pend.append(("cp", qd_, h))
                        epi_pend.append(("dma", qd_, p, Q, h))
                    del quads[(p, Q)]

            pair_tiles = {0: declare_inputs(0)}
            nc.scalar.dma_start(out=ltm, in_=ltd[:, :])

            # greedy exp routing: track projected busy-ns per engine
            load = {"act": 0.0, "dve": 0.0}

            def act_cost(fd):
                return (fd + 313) / 1.2

            def dve_cost(fd):
                return (fd + 151) / 0.96

            def emit_scores(p, Q, j, last_j, first_item, avoid=None):
                """QK matmuls + exp for one item; returns (pt, diag, t, eng)."""
                kT, qT, v2 = pair_tiles[p]
                diag = mode == "causal" and j >= 4 * Q
                t = (j - 4 * Q) * BLK if diag else 0
                st = psS.tile([BLK, 2 * QUAD], F32, tag="st")
                for h in range(2):
                    # head A always causally restricted; head B restricted only
                    # when t>=256 (where a split exp is cheaper than the extra
                    # matmul columns), else full so one exp span suffices
                    th = t if (h == 0 or t >= 256) else 0
                    if first_item:
                        # two halves so compute starts on a half-loaded chunk
                        for half in range(2):
                            nc.tensor.matmul(
                                st[:, h * QUAD + half * 256 : h * QUAD + (half + 1) * 256],
                                lhsT=kT[h * E : (h + 1) * E, j * BLK : (j + 1) * BLK],
                                rhs=qT[h * E : (h + 1) * E, half * 256 : (half + 1) * 256],
                                start=True,
                                stop=True,
                            )
                    else:
                        nc.tensor.matmul(
                            st[:, h * QUAD + th : (h + 1) * QUAD],
                            lhsT=kT[h * E : (h + 1) * E, j * BLK : (j + 1) * BLK],
                            rhs=qT[h * E : (h + 1) * E, Q * QUAD + th : (Q + 1) * QUAD],
                            start=True,
                            stop=True,
                        )
                if mode == "mask":
                    mt = stage.tile([BLK, QUAD], F32, tag="mt")
                    nc.sync.dma_start(
                        out=mt,
                        in_=maskd[j * BLK : (j + 1) * BLK, Q * QUAD : (Q + 1) * QUAD],
                    )
                    for h in range(2):
                        nc.vector.tensor_add(
                            st[:, h * QUAD : (h + 1) * QUAD],
                            st[:, h * QUAD : (h + 1) * QUAD],
                            mt,
                        )
                pt = ptp.tile([BLK, 2 * QUAD], BF16, tag="pt")
                # quad-0 diag blocks feed the short causal rows: always precise.
                forced_act = mode == "mask" or (diag and Q == 0)
                fd = 2 * QUAD - t
                if forced_act:
                    use_dve = False
                elif avoid == "act":
                    use_dve = True
                elif avoid == "dve":
                    use_dve = False
                else:
                    use_dve = load["dve"] + dve_cost(fd) < load["act"] + act_cost(fd)
                if use_dve:
                    # Schraudolph: bf16 bits = max(s'' + 15872, 0) as uint16
                    load["dve"] += dve_cost(fd)
                    nc.vector.tensor_scalar(
                        out=pt[:, t : 2 * QUAD].bitcast(U16),
                        in0=st[:, t : 2 * QUAD],
                        scalar1=DVE_B,
                        scalar2=0.0,
                        op0=mybir.AluOpType.add,
                        op1=mybir.AluOpType.max,
                    )
                elif t >= 384:
                    load["act"] += 2 * act_cost(QUAD - t)
                    for h in range(2):
                        nc.scalar.activation(
                            pt[:, h * QUAD + t : (h + 1) * QUAD],
                            st[:, h * QUAD + t : (h + 1) * QUAD],
                            mybir.ActivationFunctionType.Exp,
                            bias=bias_t[:, :],
                            scale=ACT_SCALE,
                        )
                else:
                    load["act"] += act_cost(fd)
                    nc.scalar.activation(
                        pt[:, t : 2 * QUAD],
                        st[:, t : 2 * QUAD],
                        mybir.ActivationFunctionType.Exp,
                        bias=bias_t[:, :],
                        scale=ACT_SCALE,
                    )
                return (pt, diag, t, "dve" if use_dve else "act")

            for i0 in range(0, len(items), GRP):
                batch = items[i0 : i0 + GRP]
                prev_eng = None
                for p, Q, j, last_j in batch:
                    # prefetch next pair's inputs when entering a pair
                    if (p, Q) not in quads and Q == 0 and p + 1 < NPAIR:
                        pair_tiles[p + 1] = declare_inputs(p + 1)
                    if (p, Q) not in quads:
                        otn_a = psO.tile([BLK, 4, 65], F32, tag="otA")
                        otn_b = psO.tile([BLK, 4, 65], F32, tag="otB")
                        onorm = epi.tile([BLK, 2, 4, 65], F32, tag="onorm")
                        quads[(p, Q)] = {
                            "otn": [otn_a, otn_b],
                            "onorm": onorm,
                            "started": [False, False],
                        }
                    first_item = p == 0 and Q == 0 and j == 0 and i0 == 0
                    pt, diag, t, eng = emit_scores(
                        p, Q, j, last_j, first_item, avoid=prev_eng
                    )
                    prev_eng = eng
                    av_queue.append(
                        [p, Q, j, last_j, pt, pair_tiles[p][2], diag, t, False]
                    )
                for it in av_queue[:-2]:
                    emit_mask(it)
                for _ in range(2 * len(batch)):
                    if epi_pend:
                        emit_epi(epi_pend.pop(0))
                while len(av_queue) > SKEW:
                    emit_av(av_queue.pop(0))

            for it in av_queue:
                emit_av(it)
                for _ in range(2):
                    if epi_pend:
                        emit_epi(epi_pend.pop(0))
            while epi_pend:
                emit_epi(epi_pend.pop(0))

    nc.compile()
    return nc


_programs: dict = {}


def _get_program(mode: str) -> bass.Bass:
    if mode not in _programs:
        _programs[mode] = _build(mode)
    return _programs[mode]


def _consts():
    # S^T block coords: rows=s, cols=q; valid (1.0) iff s <= q
    import ml_dtypes

    ltm = np.where(
        np.arange(BLK)[:, None] <= np.arange(BLK)[None, :], 1.0, 0.0
    ).astype(ml_dtypes.bfloat16)
    return ltm


def _prep_qkT(x_loc: np.ndarray, scale: float):
    """[L, 512] f32 -> [NPAIR, 128, L] bf16: per pair, transposed 128-col slice."""
    import ml_dtypes

    x = (x_loc * scale).astype(ml_dtypes.bfloat16)
    return np.ascontiguousarray(x.reshape(L, NPAIR, BLK).transpose(1, 2, 0))


def _prep_v2(v_loc: np.ndarray):
    """[L, 512] -> [NPAIR, 128, NBLK, 130] bf16: per pair [V_hA | ones | V_hB | ones],
    key-blocked so each DMA chunk is contiguous."""
    import ml_dtypes

    v2 = np.ones((L, NPAIR, 130), dtype=np.float32)
    v4 = v_loc.reshape(L, NPAIR, 2, E)
    v2[:, :, 0:E] = v4[:, :, 0]
    v2[:, :, 65 : 65 + E] = v4[:, :, 1]
    # [L, NPAIR, 130] -> [NPAIR, s=128, j=NBLK, 130]
    v2 = v2.reshape(NBLK, BLK, NPAIR, 130).transpose(2, 1, 0, 3)
    return np.ascontiguousarray(v2.astype(ml_dtypes.bfloat16))


def kernel(queries, keys, values, attn_mask):
    global last_exec_time_ns, last_results
    queries = np.asarray(queries, dtype=np.float32)
    keys = np.asarray(keys, dtype=np.float32)
    values = np.asarray(values, dtype=np.float32)
    attn_mask = np.asarray(attn_mask)

    causal_ref = np.triu(np.ones((L, L), dtype=bool), 1)
    m2 = attn_mask.reshape(B, L, L)
    if all(np.array_equal(m2[b], causal_ref) for b in range(B)):
        mode = "causal"
    elif not attn_mask.any():
        mode = "none"
    else:
        mode = "mask"

    trace = os.environ.get("KERNEL_TRACE", "0") == "1"
    nc = _get_program(mode)
    ltm = _consts()

    in_maps = []
    for core in range(NCORES):
        b = core // 2
        c0 = (core % 2) * DLOC
        im = {
            "qT": _prep_qkT(queries[b][:, c0 : c0 + DLOC], QSCALE),
            "kT": _prep_qkT(keys[b][:, c0 : c0 + DLOC], 1.0),
            "v2": _prep_v2(values[b][:, c0 : c0 + DLOC]),
            "ltm": ltm,
        }
        if mode == "mask":
            # kernel reads mask as [key s, query q] = transpose of [l, s]
            im["mask"] = np.ascontiguousarray(
                np.where(m2[b].T, NEG, 0.0).astype(np.float32)
            )
        in_maps.append(im)

    kw = {}
    if trace:
        kw = dict(trace=True, stitch_traces=False)
    res = run_bass_kernel_spmd(nc, in_maps, list(range(NCORES)), **kw)
    last_exec_time_ns = res.exec_time_ns
    last_results = res

    out = np.empty((B, L, D), dtype=np.float32)
    for core in range(NCORES):
        b = core // 2
        c0 = (core % 2) * DLOC
        # [NPAIR, NQUAD, 128, 2h, 4qb, 65] -> normalize -> [L, 512]
        r = res.results[core]["out"]
        r = r[..., 0:E] / r[..., 64:65]
        # q index = Q*512 + qb*128 + part ; col = (2p+h)*64 + e
        r = r.transpose(1, 4, 2, 0, 3, 5).reshape(L, DLOC)
        out[b][:, c0 : c0 + DLOC] = r
    return out


# revision 10
# speedup vs baseline: 1.0069x; 1.0069x over previous
"""Causal multi-head attention on 8 Trainium2 NeuronCores.

Problem: B=4, L=S=2048, D=1024, H=16 (E=64), fp32, causal mask.
Sharding: B x H tensor-parallel. Core k handles batch b=k//2 and heads
h in [(k%2)*8, (k%2)*8+8) -- a contiguous [2048, 512] column slice of
q/k/v. No cross-core communication.

v2 design notes (vs the fp32r baseline at ~230us):
  - QK^T in bf16: stationary gets FWL (2x faster LDWEIGHTS) and the two
    heads of a pair sit on partitions 0-63 / 64-127, so bass's implicit
    tile_position row-tiling runs both heads' matmuls concurrently.
    Host folds a 16/ln2 scale into q, so PSUM scores are s'' = 16*s/ln2.
  - exp is the real wall (ScalarE is the only table engine, 1 elem/lane/
    cycle).  Split it across two engines:
      * diagonal blocks (they alone feed the short causal rows, which
        have no averaging to hide error) run real exp on ACT:
        P = exp(s/8)/8 = exp(s''*ln2/128 - 3ln2), bf16 out.
      * ~9/16 of the off-diagonal blocks run on DVE as a Schraudolph
        bit-trick: bf16(bits) where bits = max(s'' + 15872, 0) written
        as uint16 -- one tensor_scalar per span, +-3% sawtooth error
        that averages out over long softmax rows (validated 3.4e-3
        graded err in numpy).
  - AV unchanged: P^T block stationary (bf16, FWL), rhs = [V|1] so the
    matmul also accumulates softmax row-sums in PSUM column 64.
  - Epilogue batched: one reciprocal [128,4] + one broadcast multiply
    [128,4,64] per (pair, quad, head); one contiguous [128,512] output
    DMA per (pair, quad); host reassembles the layout.
  - All DMAs are contiguous >=1KB-per-partition transfers.
"""

import os

os.environ.setdefault("MYCRO_LOCAL_CACHE", "1")

import numpy as np

import concourse.bass as bass
import concourse.mybir as mybir
import concourse.tile as tile
from concourse import bacc
from concourse.bass_utils import run_bass_kernel_spmd

F32 = mybir.dt.float32
BF16 = mybir.dt.bfloat16
U16 = mybir.dt.uint16

B, L, D, H = 4, 2048, 1024, 16
E = D // H               # 64
NCORES = 8
HLOC = H // 2            # 8 heads per core
DLOC = HLOC * E          # 512 local feature columns
NPAIR = HLOC // 2        # 4 head pairs
QUAD = 512               # q columns per PSUM tile (per head)
NQUAD = L // QUAD        # 4
BLK = 128
NBLK = L // BLK          # 16 key blocks
NEG = -3.0e7             # additive mask in s'' units; exp -> 0
LN2 = float(np.log(2.0))
QSCALE = 16.0 / LN2      # host-side q scale: scores become s'' = 16 s/ln2
ACT_SCALE = LN2 / 128.0  # exp(ACT_SCALE*s'' - 3ln2) = exp(s/8)/8
ACT_BIAS = -3.0 * LN2
DVE_B = 15872.0          # bf16 bit-space bias: 128*(127-3)=15872
SKEW = 4                 # AV deferral (items) to hide exp latency
GRP = 2                  # items per PE batch: halves QK<->AV array switches
DVE16 = int(os.environ.get("KERNEL_DVE16", "9"))  # of 16 non-diag exps on DVE

last_exec_time_ns = None
last_results = None


def _build(mode: str) -> bass.Bass:
    """mode: 'causal' | 'none' | 'mask'."""
    nc = bacc.Bacc()

    qTd = nc.declare_dram_parameter("qT", [NPAIR, BLK, L], BF16, isOutput=False)
    kTd = nc.declare_dram_parameter("kT", [NPAIR, BLK, L], BF16, isOutput=False)
    v2d = nc.declare_dram_parameter("v2", [NPAIR, BLK, NBLK, 130], BF16, isOutput=False)
    ltd = nc.declare_dram_parameter("ltm", [BLK, BLK], BF16, isOutput=False)
    if mode == "mask":
        maskd = nc.declare_dram_parameter("mask", [L, L], F32, isOutput=False)
    outd = nc.declare_dram_parameter(
        "out", [NPAIR, NQUAD, BLK, 2, 4, 65], F32, isOutput=True
    )

    with tile.TileContext(nc) as tc:
        with (
            tc.tile_pool(name="singles", bufs=1) as singles,
            tc.tile_pool(name="stage", bufs=2) as stage,   # mask staging
            tc.tile_pool(name="tbig", bufs=2) as tbig,     # qT/kT/v2
            tc.tile_pool(name="ptp", bufs=SKEW + 4) as ptp,
            tc.tile_pool(name="epi", bufs=3) as epi,
            tc.tile_pool(name="psS", bufs=2, space="PSUM") as psS,
            tc.tile_pool(name="psO", bufs=2, space="PSUM") as psO,
        ):
            ltm = singles.tile([BLK, BLK], BF16)
            bias_t = singles.tile([BLK, 1], F32)
            nc.gpsimd.memset(bias_t[:, :], ACT_BIAS)
            warm = singles.tile([BLK, 1], F32)
            nc.gpsimd.memset(warm[:, :], 0.0)
            # fires the one-time ACT exp-table load while inputs stream in
            nc.scalar.activation(
                warm[:, :], warm[:, :], mybir.ActivationFunctionType.Exp,
                bias=0.0, scale=1.0,
            )

            def declare_inputs(p):
                # chunked loads; first k chunk is small so compute starts early
                kT = tbig.tile([BLK, L], BF16, tag="kT")
                qT = tbig.tile([BLK, L], BF16, tag="qT")
                v2 = tbig.tile([BLK, NBLK, 130], BF16, tag="v2")
                nc.scalar.dma_start(out=kT[:, 0:BLK], in_=kTd[p][:, 0:BLK])
                nc.sync.dma_start(out=qT[:, 0:256], in_=qTd[p][:, 0:256])
                nc.sync.dma_start(out=qT[:, 256:QUAD], in_=qTd[p][:, 256:QUAD])
                nc.sync.dma_start(out=kT[:, BLK:QUAD], in_=kTd[p][:, BLK:QUAD])
                nc.gpsimd.dma_start(out=v2[:, 0:4, :], in_=v2d[p][:, 0:4, :])
                for c in range(1, NQUAD):
                    sl = slice(c * QUAD, (c + 1) * QUAD)
                    nc.sync.dma_start(out=kT[:, sl], in_=kTd[p][:, sl])
                    nc.sync.dma_start(out=qT[:, sl], in_=qTd[p][:, sl])
                    jl = slice(c * 4, (c + 1) * 4)
                    nc.sync.dma_start(out=v2[:, jl, :], in_=v2d[p][:, jl, :])
                return (kT, qT, v2)

            # flat work list: (pair, quad, j); within a quad, spread the
            # cheap diagonal j's among the full-width ones so per-iteration
            # PE work stays level
            items = []
            for p in range(NPAIR):
                for Q in range(NQUAD):
                    jmax = 4 * (Q + 1) if mode == "causal" else NBLK
                    if mode == "causal":
                        nond = list(range(4 * Q))
                        order = []
                        for i in range(4):
                            order += nond[i * Q : (i + 1) * Q] + [4 * Q + i]
                    else:
                        order = list(range(jmax))
                    last_j = order[-1]
                    for j in order:
                        items.append((p, Q, j, last_j))

            av_queue = []   # deferred AV work
            epi_pend = []   # deferred epilogue steps
            quads = {}      # (p, Q) -> {"otn": [...], "onorm": tile, "rsb": [...]}

            def emit_epi(step):
                kind = step[0]
                if kind == "dma":
                    _, qd_, pp, QQ, h = step
                    nc.sync.dma_start(
                        out=outd[pp][QQ][:, h], in_=qd_["onorm"][:, h]
                    )
                    return
                # "cp": move [128,4,65] (AV outputs + rowsums) PSUM->SBUF on
                # whichever engine is less loaded; host does the divide.
                _, qd_, h = step
                fd = 4 * 65
                if load["dve"] + (fd + 120) / 0.96 < load["act"] + (fd + 313) / 1.2:
                    load["dve"] += (fd + 120) / 0.96
                    nc.vector.tensor_copy(qd_["onorm"][:, h], qd_["otn"][h])
                else:
                    load["act"] += (fd + 313) / 1.2
                    nc.scalar.activation(
                        qd_["onorm"][:, h],
                        qd_["otn"][h],
                        mybir.ActivationFunctionType.Copy,
                        bias=0.0,
                        scale=1.0,
                    )

            def emit_mask(it):
                # zero the invalid triangle of both heads after exp: bf16 0/1
                # multiply in SBUF (2x DVE mode, off the st WAR path).  Emitted
                # a couple of items before the AV that consumes pt so it never
                # head-of-line-blocks fresh exps in the DVE FIFO.
                if it[6] and not it[8]:
                    pt, t = it[4], it[7]
                    load["dve"] += (BLK * 2 / 2 + 58) / 0.96
                    pt3 = pt[:, :].rearrange("s (h w) -> s h w", h=2)
                    nc.vector.tensor_tensor(
                        out=pt3[:, :, t : t + BLK],
                        in0=pt3[:, :, t : t + BLK],
                        in1=ltm[:, :].unsqueeze(1).broadcast_to([BLK, 2, BLK]),
                        op=mybir.AluOpType.mult,
                    )
                    it[8] = True

            def emit_av(it):
                emit_mask(it)
                p, Q, j, last_j, pt, v2 = it[:6]
                qd_ = quads[(p, Q)]
                qb0 = max(0, j - 4 * Q) if mode == "causal" else 0
                for h in range(2):
                    for qb in range(qb0, 4):
                        c = h * QUAD + qb * BLK
                        nc.tensor.matmul(
                            qd_["otn"][h][:, qb, :],
                            lhsT=pt[:, c : c + BLK],
                            rhs=v2[:, j, h * 65 : (h + 1) * 65],
                            start=not qd_["started"][h],
                            stop=(j == last_j and qb == 3),
                        )
                        qd_["started"][h] = True
                if j == last_j:
                    for h in range(2):
                        epi_pend.append(("cp", qd_, h))
                        epi_pend.append(("dma", qd_, p, Q, h))
                    del quads[(p, Q)]

            pair_tiles = {0: declare_inputs(0)}
            nc.scalar.dma_start(out=ltm, in_=ltd[:, :])

            # greedy exp routing: track projected busy-ns per engine
            load = {"act": 0.0, "dve": 0.0}

            def act_cost(fd):
                return (fd + 313) / 1.2

            def dve_cost(fd):
                return (fd + 151) / 0.96

            def emit_scores(p, Q, j, last_j, first_item, avoid=None):
                """QK matmuls + exp for one item; returns (pt, diag, t, eng)."""
                kT, qT, v2 = pair_tiles[p]
                diag = mode == "causal" and j >= 4 * Q
                t = (j - 4 * Q) * BLK if diag else 0
                st = psS.tile([BLK, 2 * QUAD], F32, tag="st")
                for h in range(2):
                    # head A always causally restricted; head B restricted only
                    # when t>=256 (where a split exp is cheaper than the extra
                    # matmul columns), else full so one exp span suffices
                    th = t if (h == 0 or t >= 256) else 0
                    if first_item:
                        # two halves so compute starts on a half-loaded chunk
                        for half in range(2):
                            nc.tensor.matmul(
                                st[:, h * QUAD + half * 256 : h * QUAD + (half + 1) * 256],
                                lhsT=kT[h * E : (h + 1) * E, j * BLK : (j + 1) * BLK],
                                rhs=qT[h * E : (h + 1) * E, half * 256 : (half + 1) * 256],
                                start=True,
                                stop=True,
                            )
                    else:
                        nc.tensor.matmul(
                            st[:, h * QUAD + th : (h + 1) * QUAD],
                            lhsT=kT[h * E : (h + 1) * E, j * BLK : (j + 1) * BLK],
                            rhs=qT[h * E : (h + 1) * E, Q * QUAD + th : (Q + 1) * QUAD],
                            start=True,
                            stop=True,
                        )
                if mode == "mask":
                    mt = stage.tile([BLK, QUAD], F32, tag="mt")
                    nc.sync.dma_start(
                        out=mt,
                        in_=maskd[j * BLK : (j + 1) * BLK, Q * QUAD : (Q + 1) * QUAD],
                    )
                    for h in range(2):
                        nc.vector.tensor_add(
                            st[:, h * QUAD : (h + 1) * QUAD],
                            st[:, h * QUAD : (h + 1) * QUAD],
                            mt,
                        )
                pt = ptp.tile([BLK, 2 * QUAD], BF16, tag="pt")
                # quad-0 diag blocks feed the short causal rows: always precise.
                forced_act = mode == "mask" or (diag and Q == 0)
                fd = 2 * QUAD - t
                if forced_act:
                    use_dve = False
                elif avoid == "act":
                    use_dve = True
                elif avoid == "dve":
                    use_dve = False
                else:
                    use_dve = load["dve"] + dve_cost(fd) < load["act"] + act_cost(fd)
                if use_dve:
                    # Schraudolph: bf16 bits = max(s'' + 15872, 0) as uint16
                    load["dve"] += dve_cost(fd)
                    nc.vector.tensor_scalar(
                        out=pt[:, t : 2 * QUAD].bitcast(U16),
                        in0=st[:, t : 2 * QUAD],
                        scalar1=DVE_B,
                        scalar2=0.0,
                        op0=mybir.AluOpType.add,
                        op1=mybir.AluOpType.max,
                    )
                elif t >= 384:
                    load["act"] += 2 * act_cost(QUAD - t)
                    for h in range(2):
                        nc.scalar.activation(
                            pt[:, h * QUAD + t : (h + 1) * QUAD],
                            st[:, h * QUAD + t : (h + 1) * QUAD],
                            mybir.ActivationFunctionType.Exp,
                            bias=bias_t[:, :],
                            scale=ACT_SCALE,
                        )
                else:
                    load["act"] += act_cost(fd)
                    nc.scalar.activation(
                        pt[:, t : 2 * QUAD],
                        st[:, t : 2 * QUAD],
                        mybir.ActivationFunctionType.Exp,
                        bias=bias_t[:, :],
                        scale=ACT_SCALE,
                    )
                return (pt, diag, t, "dve" if use_dve else "act")

            for i0 in range(0, len(items), GRP):
                batch = items[i0 : i0 + GRP]
                prev_eng = None
                for p, Q, j, last_j in batch:
                    # prefetch next pair's inputs when entering a pair
                    if (p, Q) not in quads and Q == 0 and p + 1 < NPAIR:
                        pair_tiles[p + 1] = declare_inputs(p + 1)
                    if (p, Q) not in quads:
                        otn_a = psO.tile([BLK, 4, 65], F32, tag="otA")
                        otn_b = psO.tile([BLK, 4, 65], F32, tag="otB")
                        onorm = epi.tile([BLK, 2, 4, 65], F32, tag="onorm")
                        quads[(p, Q)] = {
                            "otn": [otn_a, otn_b],
                            "onorm": onorm,
                            "started": [False, False],
                        }
                    first_item = p == 0 and Q == 0 and j == 0 and i0 == 0
                    pt, diag, t, eng = emit_scores(
                        p, Q, j, last_j, first_item, avoid=prev_eng
                    )
                    prev_eng = eng
                    av_queue.append(
                        [p, Q, j, last_j, pt, pair_tiles[p][2], diag, t, False]
                    )
                for it in av_queue[:-2]:
                    emit_mask(it)
                for _ in range(2 * len(batch)):
                    if epi_pend:
                        emit_epi(epi_pend.pop(0))
                while len(av_queue) > SKEW:
                    emit_av(av_queue.pop(0))

            for it in av_queue:
                emit_av(it)
                for _ in range(2):
                    if epi_pend:
                        emit_epi(epi_pend.pop(0))
            while epi_pend:
                emit_epi(epi_pend.pop(0))

    nc.compile()
    return nc


_programs: dict = {}


def _get_program(mode: str) -> bass.Bass:
    if mode not in _programs:
        _programs[mode] = _build(mode)
    return _programs[mode]


def _consts():
    # S^T block coords: rows=s, cols=q; valid (1.0) iff s <= q
    import ml_dtypes

    ltm = np.where(
        np.arange(BLK)[:, None] <= np.arange(BLK)[None, :], 1.0, 0.0
    ).astype(ml_dtypes.bfloat16)
    return ltm


def _prep_qkT(x_loc: np.ndarray, scale: float):
    """[L, 512] f32 -> [NPAIR, 128, L] bf16: per pair, transposed 128-col slice."""
    import ml_dtypes

    x = (x_loc * scale).astype(ml_dtypes.bfloat16)
    return np.ascontiguousarray(x.reshape(L, NPAIR, BLK).transpose(1, 2, 0))


def _prep_v2(v_loc: np.ndarray):
    """[L, 512] -> [NPAIR, 128, NBLK, 130] bf16: per pair [V_hA | ones | V_hB | ones],
    key-blocked so each DMA chunk is contiguous."""
    import ml_dtypes

    v2 = np.ones((L, NPAIR, 130), dtype=np.float32)
    v4 = v_loc.reshape(L, NPAIR, 2, E)
    v2[:, :, 0:E] = v4[:, :, 0]
    v2[:, :, 65 : 65 + E] = v4[:, :, 1]
    # [L, NPAIR, 130] -> [NPAIR, s=128, j=NBLK, 130]
    v2 = v2.reshape(NBLK, BLK, NPAIR, 130).transpose(2, 1, 0, 3)
    return np.ascontiguousarray(v2.astype(ml_dtypes.bfloat16))


def kernel(queries, keys, values, attn_mask):
    global last_exec_time_ns, last_results
    queries = np.asarray(queries, dtype=np.float32)
    keys = np.asarray(keys, dtype=np.float32)
    values = np.asarray(values, dtype=np.float32)
    attn_mask = np.asarray(attn_mask)

    causal_ref = np.triu(np.ones((L, L), dtype=bool), 1)
    m2 = attn_mask.reshape(B, L, L)
    if all(np.array_equal(m2[b], causal_ref) for b in range(B)):
        mode = "causal"
    elif not attn_mask.any():
        mode = "none"
    else:
        mode = "mask"

    trace = os.environ.get("KERNEL_TRACE", "0") == "1"
    nc = _get_program(mode)
    ltm = _consts()

    in_maps = []
    for core in range(NCORES):
        b = core // 2
        c0 = (core % 2) * DLOC
        im = {
            "qT": _prep_qkT(queries[b][:, c0 : c0 + DLOC], QSCALE),
            "kT": _prep_qkT(keys[b][:, c0 : c0 + DLOC], 1.0),
            "v2": _prep_v2(values[b][:, c0 : c0 + DLOC]),
            "ltm": ltm,
        }
        if mode == "mask":
            # kernel reads mask as [key s, query q] = transpose of [l, s]
            im["mask"] = np.ascontiguousarray(
                np.where(m2[b].T, NEG, 0.0).astype(np.float32)
            )
        in_maps.append(im)

    kw = {}
    if trace:
        kw = dict(trace=True, stitch_traces=False)
    res = run_bass_kernel_spmd(nc, in_maps, list(range(NCORES)), **kw)
    last_exec_time_ns = res.exec_time_ns
    last_results = res

    out = np.empty((B, L, D), dtype=np.float32)
    for core in range(NCORES):
        b = core // 2
        c0 = (core % 2) * DLOC
        # [NPAIR, NQUAD, 128, 2h, 4qb, 65] -> normalize -> [L, 512]
        r = res.results[core]["out"]
        r = r[..., 0:E] / r[..., 64:65]
        # q index = Q*512 + qb*128 + part ; col = (2p+h)*64 + e
        r = r.transpose(1, 4, 2, 0, 3, 5).reshape(L, DLOC)
        out[b][:, c0 : c0 + DLOC] = r
    return out


# revision 11
# speedup vs baseline: 1.0073x; 1.0004x over previous
"""Causal multi-head attention on 8 Trainium2 NeuronCores.

Problem: B=4, L=S=2048, D=1024, H=16 (E=64), fp32, causal mask.
Sharding: B x H tensor-parallel. Core k handles batch b=k//2 and heads
h in [(k%2)*8, (k%2)*8+8) -- a contiguous [2048, 512] column slice of
q/k/v. No cross-core communication.

v2 design notes (vs the fp32r baseline at ~230us):
  - QK^T in bf16: stationary gets FWL (2x faster LDWEIGHTS) and the two
    heads of a pair sit on partitions 0-63 / 64-127, so bass's implicit
    tile_position row-tiling runs both heads' matmuls concurrently.
    Host folds a 16/ln2 scale into q, so PSUM scores are s'' = 16*s/ln2.
  - exp is the real wall (ScalarE is the only table engine, 1 elem/lane/
    cycle).  Split it across two engines:
      * diagonal blocks (they alone feed the short causal rows, which
        have no averaging to hide error) run real exp on ACT:
        P = exp(s/8)/8 = exp(s''*ln2/128 - 3ln2), bf16 out.
      * ~9/16 of the off-diagonal blocks run on DVE as a Schraudolph
        bit-trick: bf16(bits) where bits = max(s'' + 15872, 0) written
        as uint16 -- one tensor_scalar per span, +-3% sawtooth error
        that averages out over long softmax rows (validated 3.4e-3
        graded err in numpy).
  - AV unchanged: P^T block stationary (bf16, FWL), rhs = [V|1] so the
    matmul also accumulates softmax row-sums in PSUM column 64.
  - Epilogue batched: one reciprocal [128,4] + one broadcast multiply
    [128,4,64] per (pair, quad, head); one contiguous [128,512] output
    DMA per (pair, quad); host reassembles the layout.
  - All DMAs are contiguous >=1KB-per-partition transfers.
"""

import os

os.environ.setdefault("MYCRO_LOCAL_CACHE", "1")

import numpy as np

import concourse.bass as bass
import concourse.mybir as mybir
import concourse.tile as tile
from concourse import bacc
from concourse.bass_utils import run_bass_kernel_spmd

F32 = mybir.dt.float32
BF16 = mybir.dt.bfloat16
U16 = mybir.dt.uint16

B, L, D, H = 4, 2048, 1024, 16
E = D // H               # 64
NCORES = 8
HLOC = H // 2            # 8 heads per core
DLOC = HLOC * E          # 512 local feature columns
NPAIR = HLOC // 2        # 4 head pairs
QUAD = 512               # q columns per PSUM tile (per head)
NQUAD = L // QUAD        # 4
BLK = 128
NBLK = L // BLK          # 16 key blocks
NEG = -3.0e7             # additive mask in s'' units; exp -> 0
LN2 = float(np.log(2.0))
QSCALE = 16.0 / LN2      # host-side q scale: scores become s'' = 16 s/ln2
ACT_SCALE = LN2 / 128.0  # exp(ACT_SCALE*s'' - 3ln2) = exp(s/8)/8
ACT_BIAS = -3.0 * LN2
DVE_B = 15872.0          # bf16 bit-space bias: 128*(127-3)=15872
SKEW = 4                 # AV deferral (items) to hide exp latency
GRP = 2                  # items per PE batch: halves QK<->AV array switches
DVE16 = int(os.environ.get("KERNEL_DVE16", "9"))  # of 16 non-diag exps on DVE

last_exec_time_ns = None
last_results = None


def _build(mode: str) -> bass.Bass:
    """mode: 'causal' | 'none' | 'mask'."""
    nc = bacc.Bacc()

    qTd = nc.declare_dram_parameter("qT", [NPAIR, BLK, L], BF16, isOutput=False)
    kTd = nc.declare_dram_parameter("kT", [NPAIR, BLK, L], BF16, isOutput=False)
    v2d = nc.declare_dram_parameter("v2", [NPAIR, BLK, NBLK, 130], BF16, isOutput=False)
    ltd = nc.declare_dram_parameter("ltm", [BLK, BLK], BF16, isOutput=False)
    if mode == "mask":
        maskd = nc.declare_dram_parameter("mask", [L, L], F32, isOutput=False)
    outd = nc.declare_dram_parameter(
        "out", [NPAIR, NQUAD, BLK, 2, 4, 65], BF16, isOutput=True
    )

    with tile.TileContext(nc) as tc:
        with (
            tc.tile_pool(name="singles", bufs=1) as singles,
            tc.tile_pool(name="stage", bufs=2) as stage,   # mask staging
            tc.tile_pool(name="tbig", bufs=2) as tbig,     # qT/kT/v2
            tc.tile_pool(name="ptp", bufs=SKEW + 4) as ptp,
            tc.tile_pool(name="epi", bufs=3) as epi,
            tc.tile_pool(name="psS", bufs=2, space="PSUM") as psS,
            tc.tile_pool(name="psO", bufs=2, space="PSUM") as psO,
        ):
            ltm = singles.tile([BLK, BLK], BF16)
            bias_t = singles.tile([BLK, 1], F32)
            nc.gpsimd.memset(bias_t[:, :], ACT_BIAS)
            warm = singles.tile([BLK, 1], F32)
            nc.gpsimd.memset(warm[:, :], 0.0)
            # fires the one-time ACT exp-table load while inputs stream in
            nc.scalar.activation(
                warm[:, :], warm[:, :], mybir.ActivationFunctionType.Exp,
                bias=0.0, scale=1.0,
            )

            def declare_inputs(p):
                # chunked loads; first k chunk is small so compute starts early
                kT = tbig.tile([BLK, L], BF16, tag="kT")
                qT = tbig.tile([BLK, L], BF16, tag="qT")
                v2 = tbig.tile([BLK, NBLK, 130], BF16, tag="v2")
                nc.scalar.dma_start(out=kT[:, 0:BLK], in_=kTd[p][:, 0:BLK])
                nc.sync.dma_start(out=qT[:, 0:256], in_=qTd[p][:, 0:256])
                nc.sync.dma_start(out=qT[:, 256:QUAD], in_=qTd[p][:, 256:QUAD])
                nc.sync.dma_start(out=kT[:, BLK:QUAD], in_=kTd[p][:, BLK:QUAD])
                nc.gpsimd.dma_start(out=v2[:, 0:4, :], in_=v2d[p][:, 0:4, :])
                for c in range(1, NQUAD):
                    sl = slice(c * QUAD, (c + 1) * QUAD)
                    nc.sync.dma_start(out=kT[:, sl], in_=kTd[p][:, sl])
                    nc.sync.dma_start(out=qT[:, sl], in_=qTd[p][:, sl])
                    jl = slice(c * 4, (c + 1) * 4)
                    nc.sync.dma_start(out=v2[:, jl, :], in_=v2d[p][:, jl, :])
                return (kT, qT, v2)

            # flat work list: (pair, quad, j); within a quad, spread the
            # cheap diagonal j's among the full-width ones so per-iteration
            # PE work stays level
            items = []
            for p in range(NPAIR):
                for Q in range(NQUAD):
                    jmax = 4 * (Q + 1) if mode == "causal" else NBLK
                    if mode == "causal":
                        nond = list(range(4 * Q))
                        order = []
                        for i in range(4):
                            order += nond[i * Q : (i + 1) * Q] + [4 * Q + i]
                    else:
                        order = list(range(jmax))
                    last_j = order[-1]
                    for j in order:
                        items.append((p, Q, j, last_j))

            av_queue = []   # deferred AV work
            epi_pend = []   # deferred epilogue steps
            quads = {}      # (p, Q) -> {"otn": [...], "onorm": tile, "rsb": [...]}

            def emit_epi(step):
                kind = step[0]
                if kind == "dma":
                    _, qd_, pp, QQ, h = step
                    nc.sync.dma_start(
                        out=outd[pp][QQ][:, h], in_=qd_["onorm"][:, h]
                    )
                    return
                # "cp": move [128,4,65] (AV outputs + rowsums) PSUM->SBUF on
                # whichever engine is less loaded; host does the divide.
                _, qd_, h = step
                fd = 4 * 65
                if load["dve"] + (fd / 2 + 120) / 0.96 < load["act"] + (fd + 313) / 1.2:
                    load["dve"] += (fd / 2 + 120) / 0.96
                    nc.vector.tensor_copy(qd_["onorm"][:, h], qd_["otn"][h])
                else:
                    load["act"] += (fd + 313) / 1.2
                    nc.scalar.activation(
                        qd_["onorm"][:, h],
                        qd_["otn"][h],
                        mybir.ActivationFunctionType.Copy,
                        bias=0.0,
                        scale=1.0,
                    )

            def emit_mask(it):
                # zero the invalid triangle of both heads after exp: bf16 0/1
                # multiply in SBUF (2x DVE mode, off the st WAR path).  Emitted
                # a couple of items before the AV that consumes pt so it never
                # head-of-line-blocks fresh exps in the DVE FIFO.
                if it[6] and not it[8]:
                    pt, t = it[4], it[7]
                    load["dve"] += (BLK * 2 / 2 + 58) / 0.96
                    pt3 = pt[:, :].rearrange("s (h w) -> s h w", h=2)
                    nc.vector.tensor_tensor(
                        out=pt3[:, :, t : t + BLK],
                        in0=pt3[:, :, t : t + BLK],
                        in1=ltm[:, :].unsqueeze(1).broadcast_to([BLK, 2, BLK]),
                        op=mybir.AluOpType.mult,
                    )
                    it[8] = True

            def emit_av(it):
                emit_mask(it)
                p, Q, j, last_j, pt, v2 = it[:6]
                qd_ = quads[(p, Q)]
                qb0 = max(0, j - 4 * Q) if mode == "causal" else 0
                for h in range(2):
                    for qb in range(qb0, 4):
                        c = h * QUAD + qb * BLK
                        nc.tensor.matmul(
                            qd_["otn"][h][:, qb, :],
                            lhsT=pt[:, c : c + BLK],
                            rhs=v2[:, j, h * 65 : (h + 1) * 65],
                            start=not qd_["started"][h],
                            stop=(j == last_j and qb == 3),
                        )
                        qd_["started"][h] = True
                if j == last_j:
                    for h in range(2):
                        epi_pend.append(("cp", qd_, h))
                        epi_pend.append(("dma", qd_, p, Q, h))
                    del quads[(p, Q)]

            pair_tiles = {0: declare_inputs(0)}
            nc.scalar.dma_start(out=ltm, in_=ltd[:, :])

            # greedy exp routing: track projected busy-ns per engine
            load = {"act": 0.0, "dve": 0.0}

            def act_cost(fd):
                return (fd + 313) / 1.2

            def dve_cost(fd):
                return (fd + 151) / 0.96

            def emit_scores(p, Q, j, last_j, first_item, avoid=None):
                """QK matmuls + exp for one item; returns (pt, diag, t, eng)."""
                kT, qT, v2 = pair_tiles[p]
                diag = mode == "causal" and j >= 4 * Q
                t = (j - 4 * Q) * BLK if diag else 0
                st = psS.tile([BLK, 2 * QUAD], F32, tag="st")
                for h in range(2):
                    # head A always causally restricted; head B restricted only
                    # when t>=256 (where a split exp is cheaper than the extra
                    # matmul columns), else full so one exp span suffices
                    th = t if (h == 0 or t >= 256) else 0
                    if first_item:
                        # two halves so compute starts on a half-loaded chunk
                        for half in range(2):
                            nc.tensor.matmul(
                                st[:, h * QUAD + half * 256 : h * QUAD + (half + 1) * 256],
                                lhsT=kT[h * E : (h + 1) * E, j * BLK : (j + 1) * BLK],
                                rhs=qT[h * E : (h + 1) * E, half * 256 : (half + 1) * 256],
                                start=True,
                                stop=True,
                            )
                    else:
                        nc.tensor.matmul(
                            st[:, h * QUAD + th : (h + 1) * QUAD],
                            lhsT=kT[h * E : (h + 1) * E, j * BLK : (j + 1) * BLK],
                            rhs=qT[h * E : (h + 1) * E, Q * QUAD + th : (Q + 1) * QUAD],
                            start=True,
                            stop=True,
                        )
                if mode == "mask":
                    mt = stage.tile([BLK, QUAD], F32, tag="mt")
                    nc.sync.dma_start(
                        out=mt,
                        in_=maskd[j * BLK : (j + 1) * BLK, Q * QUAD : (Q + 1) * QUAD],
                    )
                    for h in range(2):
                        nc.vector.tensor_add(
                            st[:, h * QUAD : (h + 1) * QUAD],
                            st[:, h * QUAD : (h + 1) * QUAD],
                            mt,
                        )
                pt = ptp.tile([BLK, 2 * QUAD], BF16, tag="pt")
                # quad-0 diag blocks feed the short causal rows: always precise.
                forced_act = mode == "mask" or (diag and Q == 0)
                fd = 2 * QUAD - t
                if forced_act:
                    use_dve = False
                elif avoid == "act":
                    use_dve = True
                elif avoid == "dve":
                    use_dve = False
                else:
                    use_dve = load["dve"] + dve_cost(fd) < load["act"] + act_cost(fd)
                if use_dve:
                    # Schraudolph: bf16 bits = max(s'' + 15872, 0) as uint16
                    load["dve"] += dve_cost(fd)
                    nc.vector.tensor_scalar(
                        out=pt[:, t : 2 * QUAD].bitcast(U16),
                        in0=st[:, t : 2 * QUAD],
                        scalar1=DVE_B,
                        scalar2=0.0,
                        op0=mybir.AluOpType.add,
                        op1=mybir.AluOpType.max,
                    )
                elif t >= 384:
                    load["act"] += 2 * act_cost(QUAD - t)
                    for h in range(2):
                        nc.scalar.activation(
                            pt[:, h * QUAD + t : (h + 1) * QUAD],
                            st[:, h * QUAD + t : (h + 1) * QUAD],
                            mybir.ActivationFunctionType.Exp,
                            bias=bias_t[:, :],
                            scale=ACT_SCALE,
                        )
                else:
                    load["act"] += act_cost(fd)
                    nc.scalar.activation(
                        pt[:, t : 2 * QUAD],
                        st[:, t : 2 * QUAD],
                        mybir.ActivationFunctionType.Exp,
                        bias=bias_t[:, :],
                        scale=ACT_SCALE,
                    )
                return (pt, diag, t, "dve" if use_dve else "act")

            for i0 in range(0, len(items), GRP):
                batch = items[i0 : i0 + GRP]
                prev_eng = None
                for p, Q, j, last_j in batch:
                    # prefetch next pair's inputs when entering a pair
                    if (p, Q) not in quads and Q == 0 and p + 1 < NPAIR:
                        pair_tiles[p + 1] = declare_inputs(p + 1)
                    if (p, Q) not in quads:
                        otn_a = psO.tile([BLK, 4, 65], F32, tag="otA")
                        otn_b = psO.tile([BLK, 4, 65], F32, tag="otB")
                        onorm = epi.tile([BLK, 2, 4, 65], BF16, tag="onorm")
                        quads[(p, Q)] = {
                            "otn": [otn_a, otn_b],
                            "onorm": onorm,
                            "started": [False, False],
                        }
                    first_item = p == 0 and Q == 0 and j == 0 and i0 == 0
                    pt, diag, t, eng = emit_scores(
                        p, Q, j, last_j, first_item, avoid=prev_eng
                    )
                    prev_eng = eng
                    av_queue.append(
                        [p, Q, j, last_j, pt, pair_tiles[p][2], diag, t, False]
                    )
                for it in av_queue[:-2]:
                    emit_mask(it)
                for _ in range(2 * len(batch)):
                    if epi_pend:
                        emit_epi(epi_pend.pop(0))
                while len(av_queue) > SKEW:
                    emit_av(av_queue.pop(0))

            for it in av_queue:
                emit_av(it)
                for _ in range(2):
                    if epi_pend:
                        emit_epi(epi_pend.pop(0))
            while epi_pend:
                emit_epi(epi_pend.pop(0))

    nc.compile()
    return nc


_programs: dict = {}


def _get_program(mode: str) -> bass.Bass:
    if mode not in _programs:
        _programs[mode] = _build(mode)
    return _programs[mode]


def _consts():
    # S^T block coords: rows=s, cols=q; valid (1.0) iff s <= q
    import ml_dtypes

    ltm = np.where(
        np.arange(BLK)[:, None] <= np.arange(BLK)[None, :], 1.0, 0.0
    ).astype(ml_dtypes.bfloat16)
    return ltm


def _prep_qkT(x_loc: np.ndarray, scale: float):
    """[L, 512] f32 -> [NPAIR, 128, L] bf16: per pair, transposed 128-col slice."""
    import ml_dtypes

    x = (x_loc * scale).astype(ml_dtypes.bfloat16)
    return np.ascontiguousarray(x.reshape(L, NPAIR, BLK).transpose(1, 2, 0))


def _prep_v2(v_loc: np.ndarray):
    """[L, 512] -> [NPAIR, 128, NBLK, 130] bf16: per pair [V_hA | ones | V_hB | ones],
    key-blocked so each DMA chunk is contiguous."""
    import ml_dtypes

    v2 = np.ones((L, NPAIR, 130), dtype=np.float32)
    v4 = v_loc.reshape(L, NPAIR, 2, E)
    v2[:, :, 0:E] = v4[:, :, 0]
    v2[:, :, 65 : 65 + E] = v4[:, :, 1]
    # [L, NPAIR, 130] -> [NPAIR, s=128, j=NBLK, 130]
    v2 = v2.reshape(NBLK, BLK, NPAIR, 130).transpose(2, 1, 0, 3)
    return np.ascontiguousarray(v2.astype(ml_dtypes.bfloat16))


def kernel(queries, keys, values, attn_mask):
    global last_exec_time_ns, last_results
    queries = np.asarray(queries, dtype=np.float32)
    keys = np.asarray(keys, dtype=np.float32)
    values = np.asarray(values, dtype=np.float32)
    attn_mask = np.asarray(attn_mask)

    causal_ref = np.triu(np.ones((L, L), dtype=bool), 1)
    m2 = attn_mask.reshape(B, L, L)
    if all(np.array_equal(m2[b], causal_ref) for b in range(B)):
        mode = "causal"
    elif not attn_mask.any():
        mode = "none"
    else:
        mode = "mask"

    trace = os.environ.get("KERNEL_TRACE", "0") == "1"
    nc = _get_program(mode)
    ltm = _consts()

    in_maps = []
    for core in range(NCORES):
        b = core // 2
        c0 = (core % 2) * DLOC
        im = {
            "qT": _prep_qkT(queries[b][:, c0 : c0 + DLOC], QSCALE),
            "kT": _prep_qkT(keys[b][:, c0 : c0 + DLOC], 1.0),
            "v2": _prep_v2(values[b][:, c0 : c0 + DLOC]),
            "ltm": ltm,
        }
        if mode == "mask":
            # kernel reads mask as [key s, query q] = transpose of [l, s]
            im["mask"] = np.ascontiguousarray(
                np.where(m2[b].T, NEG, 0.0).astype(np.float32)
            )
        in_maps.append(im)

    kw = {}
    if trace:
        kw = dict(trace=True, stitch_traces=False)
    res = run_bass_kernel_spmd(nc, in_maps, list(range(NCORES)), **kw)
    last_exec_time_ns = res.exec_time_ns
    last_results = res

    out = np.empty((B, L, D), dtype=np.float32)
    for core in range(NCORES):
        b = core // 2
        c0 = (core % 2) * DLOC
        # [NPAIR, NQUAD, 128, 2h, 4qb, 65] -> normalize -> [L, 512]
        r = np.asarray(res.results[core]["out"], dtype=np.float32)
        r = r[..., 0:E] / r[..., 64:65]
        # q index = Q*512 + qb*128 + part ; col = (2p+h)*64 + e
        r = r.transpose(1, 4, 2, 0, 3, 5).reshape(L, DLOC)
        out[b][:, c0 : c0 + DLOC] = r
    return out
